# revision 4
# baseline (speedup 1.0000x reference)
"""MAGNN intra-metapath aggregator on 8 TRN2 NeuronCores.

bf16 design: the kernel is DMA-bound (per-core HBM share ~360 GB/s;
the f32 version sits at 99% of that roofline), so `paths` is staged to
device DRAM as bf16 — halving HBM traffic and the roofline itself.
Output rel-err from bf16 quantization is ~1e-3 (softmax-weighted mean
over 100k instances averages the per-element noise out), far inside
the 2e-2 gate. bf16 also gives 4x PE matmul, 2x PE transpose and
2x-4x DVE throughput, keeping every compute engine well under the DMA
roofline.

Layout: uniform chunks of 500 instances = 4 blocks x 125 partitions,
one 2 MB DMA per chunk (contiguous 4 KB rows, no tail path). Per
block: 4-level bf16 add-tree on DVE gives node-sums rsum = 16*reps;
one bf16 PE transpose gives rsumT for the score matmul; scores get
LeakyReLU (DVE) + Exp (ACT, bf16 out); two accumulating PE matmuls
(alpha-numerator @ rsum, and the exp-sum) run in persistent PSUM
across all chunks. The 4 KB of per-core partials ([H, D+1]) are
combined and softmax-normalized on the host.
"""

import ml_dtypes
import numpy as np

from concourse import bacc, masks, mybir, tile
from concourse.bass_utils import run_bass_kernel_spmd

N, L, D, H = 100000, 16, 128, 8
NCORES = 8
NS = N // NCORES            # 12500 instances per core
BLK = 125                   # instances per partition-block
NB = 4                      # blocks per chunk
CHUNK = BLK * NB            # 500 instances / 2 MB per DMA
NCHUNK = NS // CHUNK        # 25 uniform chunks, no tail
FD = L * D                  # 2048
F32 = mybir.dt.float32
BF16 = mybir.dt.bfloat16
AF = mybir.ActivationFunctionType

_cached_nc = None


def _build(ns=NS, repeat=1, **_compat):
    nchunk = ns // CHUNK
    assert nchunk * CHUNK == ns
    nc = bacc.Bacc(
        "TRN2",
        target_bir_lowering=False,
        debug=False,
        enable_asserts=False,
        num_devices=NCORES,
    )
    paths_d = nc.dram_tensor("paths", [ns, L, D], BF16, kind="ExternalInput")
    tgt_d = nc.dram_tensor("target_feat", [D], F32, kind="ExternalInput")
    af_d = nc.dram_tensor("attn_fc", [H, 2 * D], F32, kind="ExternalInput")
    out_d = nc.dram_tensor("out", [H * (D + 1)], F32, kind="ExternalOutput")

    with tile.TileContext(nc) as tc:
        with (
            tc.tile_pool(name="const", bufs=1) as constp,
            tc.tile_pool(name="inp", bufs=5) as inp,
            tc.tile_pool(name="work", bufs=3) as work,
            tc.tile_pool(name="ps", bufs=1, space="PSUM") as psp,
        ):
            # ---------- constants ----------
            ident = constp.tile([128, 128], BF16)
            masks.make_identity(nc, ident[:])
            # attn_fc halves loaded TRANSPOSED via strided DMA (one-time,
            # tiny): a_tT[k, h] = attn_fc[h, k], so no setup transposes.
            a_tT = constp.tile([D, H], F32)
            nc.sync.dma_start(a_tT[:], af_d.ap()[:, 0:D].rearrange("h k -> k h"))
            a_rTf = constp.tile([D, H], F32)
            nc.sync.dma_start(
                a_rTf[:], af_d.ap()[:, D : 2 * D].rearrange("h k -> k h")
            )
            tf = constp.tile([D, 1], F32)
            nc.sync.dma_start(tf[:], tgt_d.ap().rearrange("(d one) -> d one", one=1))
            ones_row = constp.tile([1, 128], F32)
            nc.vector.memset(ones_row[:], 1.0)
            ones_col = constp.tile([128, 1], BF16)
            nc.vector.memset(ones_col[:], 1.0)
            # a_rT [D, H] in bf16, scaled by 1/L (folds the path-mean into
            # the scores)
            a_rT = constp.tile([D, H], BF16)
            nc.scalar.mul(a_rT[:], a_rTf[:, :], 1.0 / L)
            # per-head bias b[h] = a_t[h] . target  -> kept as a [1, H] row
            ps_b = psp.tile([128, H], F32, tag="setup")
            b_row = constp.tile([1, H], F32)
            nc.tensor.matmul(ps_b[:1, :H], tf[:, :1], a_tT[:, :])
            nc.vector.tensor_copy(b_row[:], ps_b[:1, :H])

            # ---------- persistent accumulators ----------
            acc_p = psp.tile([H, D], F32, tag="accP")  # sum_n w[n,h]*rsum[n,:]
            acc_s = psp.tile([H, 1], F32, tag="accS")  # sum_n w[n,h]

            paths2d = paths_d.ap().rearrange("n l d -> n (l d)")

            # ---------- main streaming loop ----------
            def do_chunk(n0, first, last):
                t = inp.tile([128, NB * FD], BF16, tag="in")
                t3 = t.rearrange("p (b f) -> p b f", b=NB)
                nc.sync.dma_start(
                    t3[:BLK, :, :],
                    paths2d[n0 : n0 + CHUNK, :].rearrange("(b p) f -> p b f", b=NB),
                )
                tmp = work.tile([128, NB * 1024], BF16, tag="tree")
                tmp3 = tmp.rearrange("p (b x) -> p b x", b=NB)
                nc.vector.tensor_add(
                    tmp3[:BLK, :, :], t3[:BLK, :, 0:1024], t3[:BLK, :, 1024:2048]
                )
                nc.vector.tensor_add(
                    tmp3[:BLK, :, 0:512], tmp3[:BLK, :, 0:512], tmp3[:BLK, :, 512:1024]
                )
                nc.vector.tensor_add(
                    tmp3[:BLK, :, 0:256], tmp3[:BLK, :, 0:256], tmp3[:BLK, :, 256:512]
                )
                nc.vector.tensor_add(
                    tmp3[:BLK, :, 0:128], tmp3[:BLK, :, 0:128], tmp3[:BLK, :, 128:256]
                )
                e_ps = psp.tile([128, NB * H], F32, tag="e", bufs=2)
                rT = work.tile([128, NB * 128], BF16, tag="rT")
                rT3 = rT.rearrange("p (b x) -> p b x", b=NB)
                for b in range(NB):
                    pt = psp.tile([128, BLK], BF16, tag="pt", bufs=2)
                    nc.tensor.transpose(
                        pt[:D, :BLK], tmp3[:BLK, b, 0:D], ident[:BLK, :BLK]
                    )
                    nc.vector.tensor_copy(rT3[:, b, :BLK], pt[:D, :BLK])
                    nc.tensor.matmul(
                        e_ps[:BLK, b * H : (b + 1) * H],
                        ones_row[:1, :BLK], b_row[:1, :],
                        start=True, stop=False, skip_group_check=True,
                    )
                    nc.tensor.matmul(
                        e_ps[:BLK, b * H : (b + 1) * H],
                        rT3[:, b, :BLK], a_rT[:, :],
                        start=False, stop=True, skip_group_check=True,
                    )
                sc = work.tile([128, NB * H], F32, tag="sc")
                nc.vector.tensor_scalar_mul(sc[:BLK, :], e_ps[:BLK, :], 0.2)
                lr = work.tile([128, NB * H], F32, tag="lr")
                nc.vector.tensor_max(lr[:BLK, :], sc[:BLK, :], e_ps[:BLK, :])
                wT = work.tile([128, NB * H], BF16, tag="wT")
                nc.scalar.activation(wT[:BLK, :], lr[:BLK, :], AF.Exp)
                wT3 = wT.rearrange("p (b h) -> p b h", b=NB)
                for b in range(NB):
                    bfirst = first and b == 0
                    blast = last and b == NB - 1
                    nc.tensor.matmul(
                        acc_p[:H, :],
                        wT3[:BLK, b, :], tmp3[:BLK, b, 0:D],
                        start=bfirst, stop=blast,
                    )
                    nc.tensor.matmul(
                        acc_s[:H, :],
                        wT3[:BLK, b, :], ones_col[:BLK, :],
                        start=bfirst, stop=blast,
                    )

            # repeat>1 is a timing-only mode (re-streams the same shard;
            # output then over-counts, never used for correctness runs)
            for r in range(repeat):
                for c in range(nchunk):
                    do_chunk(
                        c * CHUNK,
                        first=(r == 0 and c == 0),
                        last=(r == repeat - 1 and c == nchunk - 1),
                    )

            # ---------- emit per-core partial [p_raw | s] ----------
            # The 4 KB cross-core combine + softmax normalization happens on
            # the host in kernel(): cheaper than an AllReduce (~10-25 us
            # device floor) plus two DRAM bounce trips in the device tail.
            part = work.tile([H, D + 1], F32, tag="part")
            nc.vector.tensor_copy(part[:H, 0:D], acc_p[:H, :])
            nc.vector.tensor_copy(part[:H, D : D + 1], acc_s[:H, :])
            nc.sync.dma_start(
                out_d.ap().rearrange("(h d) -> h d", d=D + 1), part[:]
            )

    nc.compile()
    return nc


def kernel(target_feat, paths, attn_fc, **_unused):
    global _cached_nc
    if _cached_nc is None:
        _cached_nc = _build()
    nc = _cached_nc

    paths = np.asarray(paths, dtype=np.float32)
    shards = np.ascontiguousarray(
        paths.astype(ml_dtypes.bfloat16).reshape(NCORES, NS, L, D)
    )
    tgt = np.ascontiguousarray(np.asarray(target_feat, dtype=np.float32))
    af = np.ascontiguousarray(np.asarray(attn_fc, dtype=np.float32))
    in_maps = [
        {"paths": shards[i], "target_feat": tgt, "attn_fc": af}
        for i in range(NCORES)
    ]
    res = run_bass_kernel_spmd(nc, in_maps, core_ids=list(range(NCORES)))
    # host-side combine of the 8 per-core partials [8, D+1]
    tot = np.zeros((H, D + 1), dtype=np.float64)
    for i in range(NCORES):
        tot += np.asarray(res.results[i]["out"], dtype=np.float64).reshape(
            H, D + 1
        )
    out = tot[:, :D] / (L * tot[:, D:])
    return np.ascontiguousarray(out.reshape(H * D).astype(np.float32))


# revision 23
# speedup vs baseline: 2.3184x; 2.3184x over previous
"""MAGNN intra-metapath aggregator on 8 TRN2 NeuronCores.

bf16 design: the kernel is DMA-bound (per-core HBM share ~360 GB/s;
the f32 version sits at 99% of that roofline), so `paths` is staged to
device DRAM as bf16 — halving HBM traffic and the roofline itself.
Output rel-err from bf16 quantization is ~1.5e-3 (softmax-weighted
mean over 100k instances averages the per-element noise out), far
inside the 2e-2 gate.

Layout (every choice below measured on real HW via the hw-loop
repeat-differential):
- Uniform chunks of 512 instances = 4 blocks x 128 partitions, one
  2 MB DMA per chunk. Transfers touching <128 partitions run ~2.6x
  slower on HW, so shards are zero-padded 12500 -> 12800 and the
  padding's closed-form exp-sum contribution is subtracted on the
  host.
- Two consecutive instances per partition row ("(b p two) f" source
  pattern) -> 8 KB contiguous runs per descriptor (4 KB runs measure
  ~1.15x slower); the DMA APs are bitcast to f32 (2-byte-typed DMAs
  measure slightly slower).
- Full 4-level bf16 add-tree on DVE (PE SEQ issue at 71 ns/inst is
  the scarce resource, so PE only does 4 transposes + 1 bias + 4
  score + 4 accumulate matmuls per chunk; the exp-sum rides the
  accumulate matmul via a ones column, and LeakyReLU rides the two
  ACT exps as w = max(exp(e), exp(0.2 e))).
- Stages are software-pipelined across chunks (each engine's
  sequencer is in-order, so chunk c's tail ops issue during chunk
  c+1) and the per-core [H, D+1] partials are combined and
  softmax-normalized on the host.
"""

import ml_dtypes
import numpy as np

from concourse import bacc, masks, mybir, tile
from concourse.bass_utils import run_bass_kernel_spmd

N, L, D, H = 100000, 16, 128, 8
NCORES = 8
NS = N // NCORES            # 12500 real instances per core
NSP = 12800                 # padded: 25 uniform chunks of 512, BLK=128
# (DMA transfers touching <128 partitions run ~2.6x slower on real HW,
# so shards are zero-padded to a 128-partition-uniform layout and the
# padding's exp-sum contribution is subtracted in the host combine)
BLK = 128                   # instances per partition-block
NB = 4                      # blocks per chunk
CHUNK = BLK * NB            # 512 instances / 2 MB per DMA
NCHUNK = NSP // CHUNK       # 25 uniform chunks, no tail
FD = L * D                  # 2048
F32 = mybir.dt.float32
BF16 = mybir.dt.bfloat16
AF = mybir.ActivationFunctionType

_cached_nc = None


def _build(ns=NSP, repeat=1, paired=True, **_compat):
    nchunk = ns // CHUNK
    assert nchunk * CHUNK == ns
    nc = bacc.Bacc(
        "TRN2",
        target_bir_lowering=False,
        debug=False,
        enable_asserts=False,
        num_devices=NCORES,
    )
    paths_d = nc.dram_tensor("paths", [ns, L, D], BF16, kind="ExternalInput")
    tgt_d = nc.dram_tensor("target_feat", [D], F32, kind="ExternalInput")
    af_d = nc.dram_tensor("attn_fc", [H, 2 * D], F32, kind="ExternalInput")
    out_d = nc.dram_tensor("out", [H * (D + 1)], F32, kind="ExternalOutput")

    with tile.TileContext(nc) as tc:
        with (
            tc.tile_pool(name="const", bufs=1) as constp,
            tc.tile_pool(name="inp", bufs=6) as inp,
            tc.tile_pool(name="work", bufs=4) as work,
            tc.tile_pool(name="ps", bufs=1, space="PSUM") as psp,
        ):
            # ---------- constants ----------
            ident = constp.tile([128, 128], BF16)
            masks.make_identity(nc, ident[:])
            # attn_fc halves loaded TRANSPOSED via strided DMA (one-time,
            # tiny): a_tT[k, h] = attn_fc[h, k], so no setup transposes.
            a_tT = constp.tile([D, H], F32)
            nc.sync.dma_start(a_tT[:], af_d.ap()[:, 0:D].rearrange("h k -> k h"))
            a_rTf = constp.tile([D, H], F32)
            nc.sync.dma_start(
                a_rTf[:], af_d.ap()[:, D : 2 * D].rearrange("h k -> k h")
            )
            tf = constp.tile([D, 1], F32)
            nc.sync.dma_start(tf[:], tgt_d.ap().rearrange("(d one) -> d one", one=1))
            ones_row = constp.tile([1, 128], F32)
            nc.vector.memset(ones_row[:], 1.0)
            # a_rT [D, H] in bf16, scaled by 1/L (folds the path-mean into
            # the scores)
            a_rT = constp.tile([D, H], BF16)
            nc.scalar.mul(a_rT[:], a_rTf[:, :], 1.0 / L)
            # per-head bias b[h] = a_t[h] . target  -> kept tiled NB times
            # as a [1, NB*H] row so ONE matmul broadcasts it per chunk
            ps_b = psp.tile([128, H], F32, tag="setup")
            b_row4 = constp.tile([1, NB * H], F32)
            nc.tensor.matmul(ps_b[:1, :H], tf[:, :1], a_tT[:, :])
            for b in range(NB):
                nc.vector.tensor_copy(b_row4[:, b * H : (b + 1) * H], ps_b[:1, :H])

            # ---------- persistent accumulators ----------
            # acc_p[:, 0:D] = sum_n w[n,h]*rsum[n,:]; col D = sum_n w[n,h]
            # (the exp-sum rides along via a ones column appended to tmp)
            acc_p = psp.tile([H, D + 1], F32, tag="accP")

            paths2d = paths_d.ap().rearrange("n l d -> n (l d)")

            # ---------- main streaming loop ----------
            # Stages are SOFTWARE-PIPELINED: each engine's sequencer runs
            # in program order, so if an engine appears both early (DVE
            # tree, PE transposes) and late (DVE max, PE acc) in one
            # chunk's chain, the late op head-of-line blocks the next
            # chunk's early op and the pipeline is paced by full chain
            # latency instead of DMA. do_front issues everything through
            # the two exps; do_tail (max + acc) for chunk c is issued
            # during iteration c+1.
            def do_front_a(n0):
                t = inp.tile([128, NB * FD], BF16, tag="in")
                t3 = t.rearrange("p (b f) -> p b f", b=NB)
                if paired:
                    # two consecutive instances per partition row -> 8 KB
                    # contiguous runs per DMA descriptor. Block c = 2b+two
                    # then holds instances {n0 + b*256 + 2p + two};
                    # attention is permutation-invariant over instances so
                    # compute code does not care.
                    nc.sync.dma_start(
                        t.rearrange("p (b g) -> p b g", b=NB // 2)[
                            :BLK, :, :
                        ].bitcast(F32),
                        paths2d[n0 : n0 + CHUNK, :].rearrange(
                            "(b p two) f -> p b (two f)", b=NB // 2, two=2
                        ).bitcast(F32),
                    )
                else:
                    nc.sync.dma_start(
                        t3[:BLK, :, :],
                        paths2d[n0 : n0 + CHUNK, :].rearrange(
                            "(b p) f -> p b f", b=NB
                        ),
                    )
                # full 4-level add tree on DVE: rsum lands in tmp[:, b, 0:128]
                # (fewest PE instructions — PE SEQ issue at 71 ns/inst is the
                # scarce resource, DVE has slack)
                tmp = work.tile([128, NB * 1024], BF16, tag="tree")
                tmp3 = tmp.rearrange("p (b x) -> p b x", b=NB)
                nc.vector.tensor_add(
                    tmp3[:BLK, :, :], t3[:BLK, :, 0:1024], t3[:BLK, :, 1024:2048]
                )
                nc.vector.tensor_add(
                    tmp3[:BLK, :, 0:512], tmp3[:BLK, :, 0:512], tmp3[:BLK, :, 512:1024]
                )
                nc.vector.tensor_add(
                    tmp3[:BLK, :, 0:256], tmp3[:BLK, :, 0:256], tmp3[:BLK, :, 256:512]
                )
                nc.vector.tensor_add(
                    tmp3[:BLK, :, 0:128], tmp3[:BLK, :, 0:128], tmp3[:BLK, :, 128:256]
                )
                # ones column at col 128 so the exp-sum rides the acc matmul
                nc.vector.memset(tmp3[:BLK, :, D : D + 1], 1.0)
                # block stride padded to 128 cols: PSUM access must be
                # 4-byte aligned and BLK*2B = 250 B is not
                pt = psp.tile([128, NB * 128], BF16, tag="pt", bufs=2)
                for b in range(NB):
                    nc.tensor.transpose(
                        pt[:D, b * 128 : b * 128 + BLK],
                        tmp3[:BLK, b, 0:D],
                        ident[:BLK, :BLK],
                    )
                return tmp3, pt

            def do_front_b(st):
                tmp3, pt = st
                e_ps = psp.tile([128, NB * H], F32, tag="e", bufs=3)
                rT = work.tile([128, NB * 128], BF16, tag="rT")
                for b in range(NB):
                    nc.scalar.mul(
                        rT[:, b * 128 : b * 128 + BLK],
                        pt[:D, b * 128 : b * 128 + BLK],
                        1.0,
                    )
                nc.tensor.matmul(
                    e_ps[:BLK, :], ones_row[:1, :BLK], b_row4[:1, :],
                    start=True, stop=False, skip_group_check=True,
                )
                for b in range(NB):
                    nc.tensor.matmul(
                        e_ps[:BLK, b * H : (b + 1) * H],
                        rT[:, b * 128 : b * 128 + BLK], a_rT[:, :],
                        start=False, stop=True, skip_group_check=True,
                    )
                # w = exp(leakyrelu(e)) = max(exp(e), exp(0.2 e)) — exp is
                # monotonic, so the LeakyReLU rides the two ACT exps and the
                # DVE only does one tiny bf16 max (issued in do_tail_dve)
                w1 = work.tile([128, NB * H], BF16, tag="w1")
                nc.scalar.activation(w1[:BLK, :], e_ps[:BLK, :], AF.Exp)
                w2 = work.tile([128, NB * H], BF16, tag="w2")
                nc.scalar.activation(w2[:BLK, :], e_ps[:BLK, :], AF.Exp, scale=0.2)
                return tmp3, w1, w2

            def do_tail_dve(st):
                tmp3, w1, w2 = st
                wT = work.tile([128, NB * H], BF16, tag="wT")
                nc.vector.tensor_max(wT[:BLK, :], w1[:BLK, :], w2[:BLK, :])
                return tmp3, wT.rearrange("p (b h) -> p b h", b=NB)

            def do_tail_pe(st, first, last):
                tmp3, wT3 = st
                for b in range(NB):
                    bfirst = first and b == 0
                    blast = last and b == NB - 1
                    nc.tensor.matmul(
                        acc_p[:H, :],
                        wT3[:BLK, b, :], tmp3[:BLK, b, 0 : D + 1],
                        start=bfirst, stop=blast,
                    )

            def emit_pass():
                pend = None
                tails_done = 0
                for g in range(nchunk):
                    sa = do_front_a(g * CHUNK)
                    if pend is not None:
                        pend_t = do_tail_dve(pend)
                    sb = do_front_b(sa)
                    if pend is not None:
                        do_tail_pe(pend_t, tails_done == 0, False)
                        tails_done += 1
                    pend = sb
                pend_t = do_tail_dve(pend)
                do_tail_pe(pend_t, tails_done == 0, True)

            if repeat == 1:
                emit_pass()
            else:
                # timing-only mode: re-stream the same shard `repeat` times
                # inside a HARDWARE loop so the instruction count (and hence
                # any per-instruction host/load cost) is constant w.r.t.
                # repeat — the wall-clock slope then isolates device time.
                # acc_p restarts per iteration; output is one pass's worth.
                with tc.For_i(0, repeat):
                    emit_pass()

            # ---------- emit per-core partial [p_raw | s] ----------
            # acc_p is already fully reduced; ship the 4 KB partial. The
            # cross-core combine + softmax normalization happens on the host
            # in kernel(): cheaper than an AllReduce (~10-25 us device
            # floor) plus two DRAM bounce trips in the device tail.
            part = work.tile([H, D + 1], F32, tag="part")
            nc.vector.tensor_copy(part[:H, :], acc_p[:H, :])
            nc.sync.dma_start(
                out_d.ap().rearrange("(h d) -> h d", d=D + 1), part[:]
            )

    nc.compile()
    return nc


def make_shards(paths_f32):
    """bf16 shards zero-padded from NS=12500 to NSP=12800 per core."""
    shards = np.zeros((NCORES, NSP, L, D), dtype=ml_dtypes.bfloat16)
    shards[:, :NS] = paths_f32.astype(ml_dtypes.bfloat16).reshape(
        NCORES, NS, L, D
    )
    return shards


def kernel(target_feat, paths, attn_fc, **_unused):
    global _cached_nc
    if _cached_nc is None:
        _cached_nc = _build()
    nc = _cached_nc

    paths = np.asarray(paths, dtype=np.float32)
    tgt = np.ascontiguousarray(np.asarray(target_feat, dtype=np.float32))
    af = np.ascontiguousarray(np.asarray(attn_fc, dtype=np.float32))
    shards = make_shards(paths)
    in_maps = [
        {"paths": shards[i], "target_feat": tgt, "attn_fc": af}
        for i in range(NCORES)
    ]
    res = run_bass_kernel_spmd(nc, in_maps, core_ids=list(range(NCORES)))
    # host-side combine of the 8 per-core partials [8, D+1]
    tot = np.zeros((H, D + 1), dtype=np.float64)
    for i in range(NCORES):
        tot += np.asarray(res.results[i]["out"], dtype=np.float64).reshape(
            H, D + 1
        )
    # the NCORES*(NSP-NS) zero-padded instances have rsum = 0, so they only
    # touch the exp-sum column, each adding w0 = bf16(exp(lrelu(b_h)));
    # subtract that closed-form contribution
    b = af[:, :D].astype(np.float64) @ tgt.astype(np.float64)
    w0 = np.exp(np.maximum(b, 0.2 * b))
    w0 = w0.astype(ml_dtypes.bfloat16).astype(np.float64)
    tot[:, D] -= NCORES * (NSP - NS) * w0
    out = tot[:, :D] / (L * tot[:, D:])
    return np.ascontiguousarray(out.reshape(H * D).astype(np.float32))


# revision 30
# speedup vs baseline: 2.8065x; 1.2106x over previous
"""MAGNN intra-metapath aggregator on 8 TRN2 NeuronCores.

bf16 design: the kernel is DMA-bound (per-core HBM share ~360 GB/s;
the f32 version sits at 99% of that roofline), so `paths` is staged to
device DRAM as bf16 — halving HBM traffic and the roofline itself.
Output rel-err from bf16 quantization is ~1.5e-3 (softmax-weighted
mean over 100k instances averages the per-element noise out), far
inside the 2e-2 gate.

Layout (every choice below measured on real HW via the hw-loop
repeat-differential):
- Uniform chunks of 512 instances = 4 blocks x 128 partitions, one
  2 MB DMA per chunk. Transfers touching <128 partitions run ~2.6x
  slower on HW, so shards are zero-padded 12500 -> 12800 and the
  padding's closed-form exp-sum contribution is subtracted on the
  host.
- Two consecutive instances per partition row ("(b p two) f" source
  pattern) -> 8 KB contiguous runs per descriptor (4 KB runs measure
  ~1.15x slower); the DMA APs are bitcast to f32 (2-byte-typed DMAs
  measure slightly slower).
- Full 4-level bf16 add-tree on DVE (PE SEQ issue at 71 ns/inst is
  the scarce resource, so PE only does 4 transposes + 1 bias + 4
  score + 4 accumulate matmuls per chunk; the exp-sum rides the
  accumulate matmul via a ones column, and LeakyReLU rides the two
  ACT exps as w = max(exp(e), exp(0.2 e))).
- Stages are software-pipelined across chunks (each engine's
  sequencer is in-order, so chunk c's tail ops issue during chunk
  c+1); the last 512 instances run as 4 single-block mini-chunks so
  the post-DMA drain tail is one short chain; the per-core [H, D+1]
  partials are combined and softmax-normalized on the host.
  (NOTE: bf16 transpose-ACCUMULATE into PSUM passes the simulator but
  produces wrong results on real HW — transposes here are single-shot.)
"""

import ml_dtypes
import numpy as np

from concourse import bacc, masks, mybir, tile
from concourse.bass_utils import run_bass_kernel_spmd

N, L, D, H = 100000, 16, 128, 8
NCORES = 8
NS = N // NCORES            # 12500 real instances per core
NSP = 12800                 # padded: 25 uniform chunks of 512, BLK=128
# (DMA transfers touching <128 partitions run ~2.6x slower on real HW,
# so shards are zero-padded to a 128-partition-uniform layout and the
# padding's exp-sum contribution is subtracted in the host combine)
BLK = 128                   # instances per partition-block
NB = 4                      # blocks per chunk
CHUNK = BLK * NB            # 512 instances / 2 MB per DMA
NCHUNK = NSP // CHUNK       # 25 uniform chunks, no tail
FD = L * D                  # 2048
F32 = mybir.dt.float32
BF16 = mybir.dt.bfloat16
AF = mybir.ActivationFunctionType

_cached_nc = None


def _build(ns=NSP, repeat=1, paired=True, **_compat):
    nchunk = ns // CHUNK
    assert nchunk * CHUNK == ns
    nc = bacc.Bacc(
        "TRN2",
        target_bir_lowering=False,
        debug=False,
        enable_asserts=False,
        num_devices=NCORES,
    )
    paths_d = nc.dram_tensor("paths", [ns, L, D], BF16, kind="ExternalInput")
    tgt_d = nc.dram_tensor("target_feat", [D], F32, kind="ExternalInput")
    af_d = nc.dram_tensor("attn_fc", [H, 2 * D], F32, kind="ExternalInput")
    out_d = nc.dram_tensor("out", [H * (D + 1)], F32, kind="ExternalOutput")

    with tile.TileContext(nc) as tc:
        with (
            tc.tile_pool(name="const", bufs=1) as constp,
            tc.tile_pool(name="inp", bufs=6) as inp,
            tc.tile_pool(name="work", bufs=4) as work,
            tc.tile_pool(name="ps", bufs=1, space="PSUM") as psp,
        ):
            # ---------- constants ----------
            ident = constp.tile([128, 128], BF16)
            masks.make_identity(nc, ident[:])
            # attn_fc halves loaded TRANSPOSED via strided DMA (one-time,
            # tiny): a_tT[k, h] = attn_fc[h, k], so no setup transposes.
            a_tT = constp.tile([D, H], F32)
            nc.sync.dma_start(a_tT[:], af_d.ap()[:, 0:D].rearrange("h k -> k h"))
            a_rTf = constp.tile([D, H], F32)
            nc.sync.dma_start(
                a_rTf[:], af_d.ap()[:, D : 2 * D].rearrange("h k -> k h")
            )
            tf = constp.tile([D, 1], F32)
            nc.sync.dma_start(tf[:], tgt_d.ap().rearrange("(d one) -> d one", one=1))
            ones_row = constp.tile([1, 128], F32)
            nc.vector.memset(ones_row[:], 1.0)
            # a_rT [D, H] in bf16, scaled by 1/L (folds the path-mean into
            # the scores)
            a_rT = constp.tile([D, H], BF16)
            nc.scalar.mul(a_rT[:], a_rTf[:, :], 1.0 / L)
            # per-head bias b[h] = a_t[h] . target  -> kept tiled NB times
            # as a [1, NB*H] row so ONE matmul broadcasts it per chunk
            ps_b = psp.tile([128, H], F32, tag="setup")
            b_row4 = constp.tile([1, NB * H], F32)
            nc.tensor.matmul(ps_b[:1, :H], tf[:, :1], a_tT[:, :])
            for b in range(NB):
                nc.vector.tensor_copy(b_row4[:, b * H : (b + 1) * H], ps_b[:1, :H])

            # ---------- persistent accumulators ----------
            # acc_p[:, 0:D] = sum_n w[n,h]*rsum[n,:]; col D = sum_n w[n,h]
            # (the exp-sum rides along via a ones column appended to tmp)
            acc_p = psp.tile([H, D + 1], F32, tag="accP")

            paths2d = paths_d.ap().rearrange("n l d -> n (l d)")

            # ---------- main streaming loop ----------
            # Stages are SOFTWARE-PIPELINED: each engine's sequencer runs
            # in program order, so if an engine appears both early (DVE
            # tree, PE transposes) and late (DVE max, PE acc) in one
            # chunk's chain, the late op head-of-line blocks the next
            # chunk's early op and the pipeline is paced by full chain
            # latency instead of DMA. do_front issues everything through
            # the two exps; do_tail (max + acc) for chunk c is issued
            # during iteration c+1.
            def do_front_a(n0, nb):
                t = inp.tile([128, nb * FD], BF16, tag=f"in{nb}")
                t3 = t.rearrange("p (b f) -> p b f", b=nb)
                if paired and nb % 2 == 0:
                    # two consecutive instances per partition row -> 8 KB
                    # contiguous runs per DMA descriptor. Block c = 2b+two
                    # then holds instances {n0 + b*256 + 2p + two};
                    # attention is permutation-invariant over instances so
                    # compute code does not care.
                    nc.sync.dma_start(
                        t.rearrange("p (b g) -> p b g", b=nb // 2)[
                            :BLK, :, :
                        ].bitcast(F32),
                        paths2d[n0 : n0 + nb * BLK, :].rearrange(
                            "(b p two) f -> p b (two f)", b=nb // 2, two=2
                        ).bitcast(F32),
                    )
                else:
                    nc.sync.dma_start(
                        t3[:BLK, :, :].bitcast(F32),
                        paths2d[n0 : n0 + nb * BLK, :].rearrange(
                            "(b p) f -> p b f", b=nb
                        ).bitcast(F32),
                    )
                # full 4-level add tree on DVE: rsum lands in tmp[:, b, 0:128]
                # (fewest PE instructions — PE SEQ issue at 71 ns/inst is the
                # scarce resource, DVE has slack)
                tmp = work.tile([128, nb * 1024], BF16, tag=f"tree{nb}")
                tmp3 = tmp.rearrange("p (b x) -> p b x", b=nb)
                nc.vector.tensor_add(
                    tmp3[:BLK, :, :], t3[:BLK, :, 0:1024], t3[:BLK, :, 1024:2048]
                )
                nc.vector.tensor_add(
                    tmp3[:BLK, :, 0:512], tmp3[:BLK, :, 0:512], tmp3[:BLK, :, 512:1024]
                )
                nc.vector.tensor_add(
                    tmp3[:BLK, :, 0:256], tmp3[:BLK, :, 0:256], tmp3[:BLK, :, 256:512]
                )
                nc.vector.tensor_add(
                    tmp3[:BLK, :, 0:128], tmp3[:BLK, :, 0:128], tmp3[:BLK, :, 128:256]
                )
                # ones column at col 128 so the exp-sum rides the acc matmul
                nc.vector.memset(tmp3[:BLK, :, D : D + 1], 1.0)
                # block stride padded to 128 cols: PSUM access must be
                # 4-byte aligned and BLK*2B = 250 B is not
                pt = psp.tile([128, NB * 128], BF16, tag="pt", bufs=2)
                for b in range(nb):
                    nc.tensor.transpose(
                        pt[:D, b * 128 : b * 128 + BLK],
                        tmp3[:BLK, b, 0:D],
                        ident[:BLK, :BLK],
                    )
                return tmp3, pt, nb

            def do_front_b(st):
                tmp3, pt, nb = st
                e_ps = psp.tile([128, NB * H], F32, tag="e", bufs=3)
                rT = work.tile([128, NB * 128], BF16, tag="rT")
                for b in range(nb):
                    nc.scalar.mul(
                        rT[:, b * 128 : b * 128 + BLK],
                        pt[:D, b * 128 : b * 128 + BLK],
                        1.0,
                    )
                nc.tensor.matmul(
                    e_ps[:BLK, 0 : nb * H],
                    ones_row[:1, :BLK], b_row4[:1, 0 : nb * H],
                    start=True, stop=False, skip_group_check=True,
                )
                for b in range(nb):
                    nc.tensor.matmul(
                        e_ps[:BLK, b * H : (b + 1) * H],
                        rT[:, b * 128 : b * 128 + BLK], a_rT[:, :],
                        start=False, stop=True, skip_group_check=True,
                    )
                # w = exp(leakyrelu(e)) = max(exp(e), exp(0.2 e)) — exp is
                # monotonic, so the LeakyReLU rides the two ACT exps and the
                # DVE only does one tiny bf16 max (issued in do_tail_dve)
                w1 = work.tile([128, NB * H], BF16, tag="w1")
                nc.scalar.activation(
                    w1[:BLK, 0 : nb * H], e_ps[:BLK, 0 : nb * H], AF.Exp
                )
                w2 = work.tile([128, NB * H], BF16, tag="w2")
                nc.scalar.activation(
                    w2[:BLK, 0 : nb * H], e_ps[:BLK, 0 : nb * H], AF.Exp, scale=0.2
                )
                return tmp3, w1, w2, nb

            def do_tail_dve(st):
                tmp3, w1, w2, nb = st
                wT = work.tile([128, NB * H], BF16, tag="wT")
                nc.vector.tensor_max(
                    wT[:BLK, 0 : nb * H], w1[:BLK, 0 : nb * H], w2[:BLK, 0 : nb * H]
                )
                return tmp3, wT.rearrange("p (b h) -> p b h", b=NB), nb

            def do_tail_pe(st, first, last):
                tmp3, wT3, nb = st
                for b in range(nb):
                    bfirst = first and b == 0
                    blast = last and b == nb - 1
                    nc.tensor.matmul(
                        acc_p[:H, :],
                        wT3[:BLK, b, :], tmp3[:BLK, b, 0 : D + 1],
                        start=bfirst, stop=blast,
                    )

            # last 512 instances go as 4 single-block minis so the drain
            # tail after the final DMA is one short chain, not a full chunk
            nbig = max(nchunk - 1, 0)
            rem = ns - nbig * CHUNK
            chunk_list = [(c * CHUNK, NB) for c in range(nbig)] + [
                (nbig * CHUNK + i * BLK, 1) for i in range(rem // BLK)
            ]

            def emit_pass():
                pend = None
                tails_done = 0
                for n0, nb in chunk_list:
                    sa = do_front_a(n0, nb)
                    if pend is not None:
                        pend_t = do_tail_dve(pend)
                    sb = do_front_b(sa)
                    if pend is not None:
                        do_tail_pe(pend_t, tails_done == 0, False)
                        tails_done += 1
                    pend = sb
                pend_t = do_tail_dve(pend)
                do_tail_pe(pend_t, tails_done == 0, True)

            if repeat == 1:
                emit_pass()
            else:
                # timing-only mode: re-stream the same shard `repeat` times
                # inside a HARDWARE loop so the instruction count (and hence
                # any per-instruction host/load cost) is constant w.r.t.
                # repeat — the wall-clock slope then isolates device time.
                # acc_p restarts per iteration; output is one pass's worth.
                with tc.For_i(0, repeat):
                    emit_pass()

            # ---------- emit per-core partial [p_raw | s] ----------
            # acc_p is already fully reduced; ship the 4 KB partial. The
            # cross-core combine + softmax normalization happens on the host
            # in kernel(): cheaper than an AllReduce (~10-25 us device
            # floor) plus two DRAM bounce trips in the device tail.
            part = work.tile([H, D + 1], F32, tag="part")
            nc.vector.tensor_copy(part[:H, :], acc_p[:H, :])
            nc.sync.dma_start(
                out_d.ap().rearrange("(h d) -> h d", d=D + 1), part[:]
            )

    nc.compile()
    return nc


def make_shards(paths_f32):
    """bf16 shards zero-padded from NS=12500 to NSP=12800 per core."""
    shards = np.zeros((NCORES, NSP, L, D), dtype=ml_dtypes.bfloat16)
    shards[:, :NS] = paths_f32.astype(ml_dtypes.bfloat16).reshape(
        NCORES, NS, L, D
    )
    return shards


def kernel(target_feat, paths, attn_fc, **_unused):
    global _cached_nc
    if _cached_nc is None:
        _cached_nc = _build()
    nc = _cached_nc

    paths = np.asarray(paths, dtype=np.float32)
    tgt = np.ascontiguousarray(np.asarray(target_feat, dtype=np.float32))
    af = np.ascontiguousarray(np.asarray(attn_fc, dtype=np.float32))
    shards = make_shards(paths)
    in_maps = [
        {"paths": shards[i], "target_feat": tgt, "attn_fc": af}
        for i in range(NCORES)
    ]
    res = run_bass_kernel_spmd(nc, in_maps, core_ids=list(range(NCORES)))
    # host-side combine of the 8 per-core partials [8, D+1]
    tot = np.zeros((H, D + 1), dtype=np.float64)
    for i in range(NCORES):
        tot += np.asarray(res.results[i]["out"], dtype=np.float64).reshape(
            H, D + 1
        )
    # the NCORES*(NSP-NS) zero-padded instances have rsum = 0, so they only
    # touch the exp-sum column, each adding w0 = bf16(exp(lrelu(b_h)));
    # subtract that closed-form contribution
    b = af[:, :D].astype(np.float64) @ tgt.astype(np.float64)
    w0 = np.exp(np.maximum(b, 0.2 * b))
    w0 = w0.astype(ml_dtypes.bfloat16).astype(np.float64)
    tot[:, D] -= NCORES * (NSP - NS) * w0
    out = tot[:, :D] / (L * tot[:, D:])
    return np.ascontiguousarray(out.reshape(H * D).astype(np.float32))
